# revision 14
# baseline (speedup 1.0000x reference)
"""Trainium2 Bass kernel for nn_DimNet (4D-conv net + pixel shuffle).

Math: the three 4D convs collapse to 2D convs over flattened angular dims:
  conv1:  in [25, 104, 104] -> out [400, 96, 96], 9x9 kernel
  conv2a: in [25, 104, 104] -> buf [180, 100, 100] (20ch x 3x3 angular window, 5x5)
  conv2b: buf [180,100,100] -> out [400, 96, 96], 5x5 kernel
  mid = (p1 + relu-path)/2; pixel-shuffle (host side, pure data movement)

Mapping to TensorE: contraction K packed as (channel, kh-shift) on partitions
(125/128-wide), kw handled by free-dim offsets into kh-shifted input copies,
accumulated in PSUM across kw / K-chunks.

Sharding: batch (2) x spatial row-slab (4 x 24 out rows) = 8 cores. Each core
computes ALL 400 output channels (4 M-chunks of 100) for its 24 rows; conv2a
is computed only for the 28 rows the slab needs (vs 100 replicated rows in
the channel-sharded layout). Gather + pixel shuffle on host.

conv2b is split as sigma 0..127 (kh,kw via free-dim offsets into buf1; 25
matmuls at K=128) + sigma 128..179 repacked (kh,kw)-dense into 13 tiles of
two 52-lane groups each (13 matmuls), built by 25 DVE scatter copies that
overlap the conv1 phase. 38 matmuls/block vs 40 in the kh-packed layout.
"""

import os
import time

import ml_dtypes
import numpy as np

import concourse.tile as tile
from concourse import bacc, mybir

F32 = mybir.dt.float32
BF16 = mybir.dt.bfloat16

MM_DT = BF16
MM_NP = ml_dtypes.bfloat16

B = 2
H = 96
W = 96
HP = H + 8   # 104
WP = W + 8   # 104
SLAB = 24    # output rows per core
SROWS = SLAB + 4  # conv2a rows a slab needs

_RUNNERS = {}


def _build_nc(reps=1):
    nc = bacc.Bacc("TRN2", target_bir_lowering=False, debug=False,
                   enable_asserts=True, num_devices=8)

    xk1 = nc.dram_tensor("xk1", [125, SROWS, WP], MM_DT, kind="ExternalInput").ap()
    xk2 = nc.dram_tensor("xk2", [100, SLAB, WP], MM_DT, kind="ExternalInput").ap()
    w1a = nc.dram_tensor("w1a", [125, 9, 400], MM_DT, kind="ExternalInput").ap()
    w1b = nc.dram_tensor("w1b", [100, 9, 400], MM_DT, kind="ExternalInput").ap()
    w2a = nc.dram_tensor("w2a", [125, 5, 180], MM_DT, kind="ExternalInput").ap()
    w2b1 = nc.dram_tensor("w2b1", [128, 25, 400], MM_DT, kind="ExternalInput").ap()
    # conv2b sigma-chunk2 (52 wide) repacked with (kh,kw) dense: tile t holds
    # khw groups 2t (lanes 0-51) and 2t+1 (lanes 64-115, 64-aligned so DVE
    # partition bases stay 32-aligned); pad lanes have zero weights.
    # 25 khw groups -> 13 tiles -> 13 matmuls (vs 15 kh-packed).
    w2bk = nc.dram_tensor("w2bk", [13, 128, 400], MM_DT, kind="ExternalInput").ap()
    ba1 = nc.dram_tensor("ba1", [128, 1], F32, kind="ExternalInput").ap()
    ba2 = nc.dram_tensor("ba2", [52, 1], F32, kind="ExternalInput").ap()
    b1h = nc.dram_tensor("b1h", [100, 4], F32, kind="ExternalInput").ap()
    b2bh = nc.dram_tensor("b2bh", [100, 4], F32, kind="ExternalInput").ap()
    # one output per rep so no rep's work is dead (reps>1 is timing-only)
    outs_d = [nc.dram_tensor("out" if r == 0 else f"out{r}",
                             [4, 100, SLAB, W], F32,
                             kind="ExternalOutput").ap() for r in range(reps)]

    Relu = mybir.ActivationFunctionType.Relu
    Add = mybir.AluOpType.add

    def mm(out, lhsT, rhs, start, stop):
        nc.tensor.matmul(out, lhsT, rhs, start=start, stop=stop)

    from contextlib import ExitStack

    with tile.TileContext(nc) as tc:
        with (
            tc.tile_pool(name="const", bufs=1) as const,
            tc.tile_pool(name="tmp", bufs=3) as tmp,
            tc.tile_pool(name="outp", bufs=3) as outp,
        ):
          # weights/biases loaded once (shared across timing reps); ordered so
          # the tensors needed first arrive first (conv2a -> conv1 -> conv2b)
          w2a_t = const.tile([125, 5, 180], MM_DT)
          w1a_t = const.tile([125, 9, 400], MM_DT)
          w1b_t = const.tile([100, 9, 400], MM_DT)
          w2b1_t = const.tile([128, 25, 400], MM_DT)
          w2bk_t = [const.tile([128, 400], MM_DT, name=f"w2bk{t}",
                               tag=f"w2bk{t}") for t in range(13)]
          ba1_t = const.tile([128, 1], F32)
          ba2_t = const.tile([52, 1], F32)
          b1h_t = const.tile([100, 4], F32)
          b2bh_t = const.tile([100, 4], F32)
          for t, src in ((w2a_t, w2a), (ba1_t, ba1), (ba2_t, ba2),
                         (w1a_t, w1a), (w1b_t, w1b), (b1h_t, b1h),
                         (w2b1_t, w2b1), (b2bh_t, b2bh)):
              nc.gpsimd.dma_start(out=t[:], in_=src)
          for t in range(13):
              nc.gpsimd.dma_start(out=w2bk_t[t][:], in_=w2bk[t])

          # (kh,kw)-shifted copies of buf2: tile t lane j*52+s' holds
          # buf2[s', l+kh, w+kw] for khw group 2t+j. Pad lanes zeroed once
          # -- 0 * garbage could be NaN.
          bk3_t = [const.tile([128, SLAB, W], MM_DT, name=f"bk3_{t}",
                              tag=f"bk3_{t}") for t in range(13)]
          for bt in bk3_t:
              nc.gpsimd.memset(bt[:], 0.0)

          for _rep in range(reps):
            out = outs_d[_rep]
            _ph_a = ExitStack()
            xk1p = _ph_a.enter_context(
                tc.tile_pool(name=f"xk1p{_rep}", bufs=2))
            xk2p = _ph_a.enter_context(
                tc.tile_pool(name=f"xk2p{_rep}", bufs=2))
            psa = _ph_a.enter_context(
                tc.tile_pool(name=f"psa{_rep}", bufs=3, space="PSUM"))
            ps1p = _ph_a.enter_context(
                tc.tile_pool(name=f"ps1{_rep}", bufs=3, space="PSUM"))
            buf1_t = const.tile([128, SROWS, 100], MM_DT)
            buf2_t = const.tile([52, SROWS, 100], MM_DT)
            p1h_t = const.tile([100, 4, SLAB, W], F32)

            # whole input slab in one DMA each (1.25 MB total)
            xc1 = xk1p.tile([125, SROWS, WP], MM_DT)
            nc.sync.dma_start(out=xc1[:], in_=xk1)
            xc2 = xk2p.tile([100, SLAB, WP], MM_DT)
            nc.sync.dma_start(out=xc2[:], in_=xk2)

            # ---- Phase A1: conv2a over the 28 slab rows (blocks of 5+3)
            for l0 in range(0, SROWS, 5):
                rb = min(5, SROWS - l0)
                pa1 = psa.tile([128, 5, 100], F32, tag="pa")
                for kw in range(5):
                    mm(pa1[:, 0:rb, :], w2a_t[:, kw, 0:128],
                       xc1[:, l0:l0 + rb, kw:kw + 100],
                       start=(kw == 0), stop=(kw == 4))
                nc.scalar.activation(buf1_t[:, l0:l0 + rb, :], pa1[:, 0:rb, :],
                                     Relu, bias=ba1_t[:])
                pa2 = psa.tile([52, 5, 100], F32, tag="pa")
                for kw in range(5):
                    mm(pa2[:, 0:rb, :], w2a_t[:, kw, 128:180],
                       xc1[:, l0:l0 + rb, kw:kw + 100],
                       start=(kw == 0), stop=(kw == 4))
                # chunk2 relu on DVE so ACT isn't the drain bottleneck
                nc.vector.tensor_scalar(buf2_t[:, l0:l0 + rb, :],
                                        pa2[:, 0:rb, :],
                                        ba2_t[:], 0.0, Add,
                                        mybir.AluOpType.max)

            # ---- Phase A2: conv1, 4 M-chunks x blocks (5,5,5,5,4); the
            # 25 bk3 scatter copies (DVE) are spread between M-chunks so
            # they overlap conv1 matmuls without starving the ps1 drains
            for mc in range(4):
                co0 = mc * 100
                for g in range(mc * 6 + (mc > 0), min((mc + 1) * 6 + 1, 25)):
                    kh, kw = divmod(g, 5)
                    t, j = divmod(g, 2)
                    nc.vector.tensor_copy(
                        bk3_t[t][j * 64:j * 64 + 52, :, :],
                        buf2_t[:, kh:kh + SLAB, kw:kw + W])
                for l0 in range(0, SLAB, 5):
                    rb = min(5, SLAB - l0)
                    p1 = ps1p.tile([100, 5, W], F32, tag="p1")
                    for kw in range(9):
                        mm(p1[:, 0:rb, :], w1a_t[:, kw, co0:co0 + 100],
                           xc1[:, l0:l0 + rb, kw:kw + W],
                           start=(kw == 0), stop=False)
                    for kw in range(9):
                        mm(p1[:, 0:rb, :], w1b_t[:, kw, co0:co0 + 100],
                           xc2[:, l0:l0 + rb, kw:kw + W],
                           start=False, stop=(kw == 8))
                    # w1/b1 pre-halved on host: p1h = psum + b1h
                    nc.vector.tensor_scalar_add(
                        p1h_t[:, mc, l0:l0 + rb, :], p1[:, 0:rb, :],
                        b1h_t[:, mc:mc + 1])

            # phase-A psum/xk pools released -> conv2b gets 6 PSUM banks
            _ph_a.close()
            ps2p = ExitStack()
            ps2 = ps2p.enter_context(
                tc.tile_pool(name=f"ps2{_rep}", bufs=6, space="PSUM"))

            # ---- Phase B: conv2b, 4 M-chunks x blocks (5,5,5,5,4) + merge
            for mc in range(4):
                co0 = mc * 100
                for l0 in range(0, SLAB, 5):
                    rb = min(5, SLAB - l0)
                    p2 = ps2.tile([100, 5, W], F32, tag="p2")
                    p2v = p2[:, 0:rb, :]
                    first = True
                    for kh in range(5):
                        for kw in range(5):
                            mm(p2v, w2b1_t[:, kh * 5 + kw, co0:co0 + 100],
                               buf1_t[:, l0 + kh:l0 + kh + rb, kw:kw + W],
                               start=first, stop=False)
                            first = False
                    for t in range(13):
                        mm(p2v, w2bk_t[t][:, co0:co0 + 100],
                           bk3_t[t][:, l0:l0 + rb, :],
                           start=False, stop=(t == 12))
                    # w2b/b2b pre-halved on host:
                    # relu(conv2b+b2b)/2 = relu(psum+b2bh)
                    tt = tmp.tile([100, 5, W], F32)
                    nc.scalar.activation(tt[:, 0:rb, :], p2v, Relu,
                                         bias=b2bh_t[:, mc:mc + 1])
                    ot = outp.tile([100, 5, W], F32)
                    nc.vector.tensor_add(ot[:, 0:rb, :], tt[:, 0:rb, :],
                                         p1h_t[:, mc, l0:l0 + rb, :])
                    nc.scalar.dma_start(out=out[mc, :, l0:l0 + rb, :],
                                        in_=ot[:, 0:rb, :])
            ps2p.close()

    nc.compile()
    return nc


def _w2bk13(W2B):
    """[13, 128, 400] (kh,kw)-dense repack of W2B[128:]: tile t lane j*52+s'
    = W2B[128+s', khw=2t+j, :]; pad lanes zero."""
    src = W2B[128:]  # [52, 25, 400]
    out = np.zeros((13, 128, 400), dtype=np.float32)
    for g in range(25):
        t, j = divmod(g, 2)
        out[t, j * 64:j * 64 + 52] = src[:, g, :]
    return np.ascontiguousarray(out.astype(MM_NP))


def _prep_in_maps(pic, w1, b1, w2a, b2a, w2b, b2b):
    pic = np.asarray(pic, dtype=np.float32).reshape(B, 25, H, W)
    w1r = np.asarray(w1, dtype=np.float32).reshape(400, 25, 9, 9)
    b1 = np.asarray(b1, dtype=np.float32)
    w2a = np.asarray(w2a, dtype=np.float32)
    b2a = np.asarray(b2a, dtype=np.float32)
    w2b = np.asarray(w2b, dtype=np.float32)
    b2b = np.asarray(b2b, dtype=np.float32)

    xpad = np.full((B, 25, HP, WP), 0.5, dtype=np.float32)
    xpad[:, :, 4:4 + H, 4:4 + W] = pic
    # xk1[b, cin*5+kh, r, w] = xpad[b, cin, r+kh, w]   (kh 0..4, r 0..99)
    xk1 = np.stack([xpad[:, :, kh:kh + 100, :] for kh in range(5)],
                   axis=2).reshape(B, 125, 100, WP)
    # xk2[b, cin*4+kh', h, w] = xpad[b, cin, h+5+kh', w] (kh' 0..3, h 0..95)
    xk2 = np.stack([xpad[:, :, 5 + kh:5 + kh + 96, :] for kh in range(4)],
                   axis=2).reshape(B, 100, 96, WP)
    xk1 = xk1.astype(MM_NP)
    xk2 = xk2.astype(MM_NP)

    # W2A[p=(a1*5+a2)*5+kh, kw, m=a1'*60+a2'*20+c] = w2a[c,0,da1,da2,kh,kw]
    W2A = np.zeros((125, 5, 180), dtype=np.float32)
    for a1p in range(3):
        for a2p in range(3):
            m0 = a1p * 60 + a2p * 20
            for da1 in range(3):
                for da2 in range(3):
                    p0 = ((a1p + da1) * 5 + (a2p + da2)) * 5
                    W2A[p0:p0 + 5, :, m0:m0 + 20] = np.transpose(
                        w2a[:, 0, da1, da2, :, :], (1, 2, 0))
    ba_full = np.tile(b2a, 9).astype(np.float32)[:, None]  # [180,1]
    W2A = np.ascontiguousarray(W2A.astype(MM_NP))

    # weights shared by all cores; w1, w2b (and biases) pre-scaled by 0.5 so
    # the (p1+p2)/2 average is folded into the matmuls.
    w1s = 0.5 * w1r  # [400, 25, 9, 9]
    W1A = np.ascontiguousarray(
        np.transpose(w1s[:, :, 0:5, :], (1, 2, 3, 0)).reshape(125, 9, 400)
    ).astype(MM_NP)
    W1B = np.ascontiguousarray(
        np.transpose(w1s[:, :, 5:9, :], (1, 2, 3, 0)).reshape(100, 9, 400)
    ).astype(MM_NP)
    w2bs = 0.5 * w2b  # [400, 20, 3, 3, 5, 5]
    W2B = np.ascontiguousarray(
        np.transpose(w2bs, (2, 3, 1, 4, 5, 0)).reshape(180, 25, 400))
    w2b1_h = np.ascontiguousarray(W2B[:128].astype(MM_NP))
    w2bk_h = _w2bk13(W2B)
    ba1_h = np.ascontiguousarray(ba_full[:128])
    ba2_h = np.ascontiguousarray(ba_full[128:])
    b1h_h = np.ascontiguousarray((0.5 * b1).reshape(4, 100).T)
    b2bh_h = np.ascontiguousarray((0.5 * b2b).reshape(4, 100).T)

    in_maps = []
    for core in range(8):
        b, sl = divmod(core, 4)
        h0 = sl * SLAB
        in_maps.append({
            "xk1": np.ascontiguousarray(xk1[b][:, h0:h0 + SROWS, :]),
            "xk2": np.ascontiguousarray(xk2[b][:, h0:h0 + SLAB, :]),
            "w1a": W1A,
            "w1b": W1B,
            "w2a": W2A,
            "w2b1": w2b1_h,
            "w2bk": w2bk_h,
            "ba1": ba1_h,
            "ba2": ba2_h,
            "b1h": b1h_h,
            "b2bh": b2bh_h,
        })
    return in_maps


def _get_runner(reps=1):
    """Build nc once per reps and return a cached jitted SPMD executor."""
    if reps in _RUNNERS:
        return _RUNNERS[reps]

    import jax
    from jax.experimental.shard_map import shard_map
    from jax.sharding import Mesh, NamedSharding, PartitionSpec

    from concourse import mybir as _mybir
    from concourse.bass2jax import (_bass_exec_p, install_neuronx_cc_hook,
                                    partition_id_tensor)

    nc = _build_nc(reps)
    install_neuronx_cc_hook()

    n_cores = 8
    partition_name = (nc.partition_id_tensor.name
                      if nc.partition_id_tensor else None)
    in_names, out_names, out_avals, zero_outs = [], [], [], []
    for alloc in nc.m.functions[0].allocations:
        if not isinstance(alloc, _mybir.MemoryLocationSet):
            continue
        name = alloc.memorylocations[0].name
        if alloc.kind == "ExternalInput":
            if name != partition_name:
                in_names.append(name)
        elif alloc.kind == "ExternalOutput":
            shape = tuple(alloc.tensor_shape)
            dtype = _mybir.dt.np(alloc.dtype)
            out_names.append(name)
            out_avals.append(jax.core.ShapedArray(shape, dtype))
            zero_outs.append(np.zeros((n_cores * shape[0],) + shape[1:], dtype))
    assert nc.dbg_addr is None
    n_params = len(in_names)
    all_names = in_names + out_names
    if partition_name is not None:
        all_names = all_names + [partition_name]

    def _body(*args):
        operands = list(args)
        if partition_name is not None:
            operands.append(partition_id_tensor())
        outs = _bass_exec_p.bind(
            *operands,
            out_avals=tuple(out_avals),
            in_names=tuple(all_names),
            out_names=tuple(out_names),
            lowering_input_output_aliases=(),
            sim_require_finite=True,
            sim_require_nnan=True,
            nc=nc,
        )
        return tuple(outs)

    devices = jax.devices()[:n_cores]
    mesh = Mesh(np.asarray(devices), ("core",))
    nspec = (PartitionSpec("core"),) * (n_params + len(out_names))
    sharded = jax.jit(
        shard_map(_body, mesh=mesh, in_specs=nspec,
                  out_specs=(PartitionSpec("core"),) * len(out_names)),
        keep_unused=True)
    sharding = NamedSharding(mesh, PartitionSpec("core"))

    class Runner:
        def put(self, in_maps):
            """Transfer inputs (+ zero output bufs) to the devices once."""
            concat_in = [
                np.concatenate([np.asarray(m[name]) for m in in_maps], axis=0)
                for name in in_names
            ]
            return [jax.device_put(x, sharding)
                    for x in concat_in + zero_outs]

        def exec_timed(self, dev_args):
            t0 = time.perf_counter()
            out_arrs = sharded(*dev_args)
            # one sync only: under axon each block_until_ready is a costly
            # RPC, and blocking any output waits for the whole execution
            out_arrs[0].block_until_ready()
            return out_arrs, time.perf_counter() - t0

        def dispatch(self, dev_args):
            """Async dispatch without blocking (for pipelined timing)."""
            return sharded(*dev_args)

        def __call__(self, in_maps):
            out_arrs, dt = self.exec_timed(self.put(in_maps))
            per_core = [
                {name: np.asarray(out_arrs[i]).reshape(
                    n_cores, *out_avals[i].shape)[c]
                 for i, name in enumerate(out_names)}
                for c in range(n_cores)
            ]
            return per_core, dt

    run = Runner()
    _RUNNERS[reps] = run
    return run


def kernel(pic, w1, b1, w2a, b2a, w2b, b2b):
    run = _get_runner()
    in_maps = _prep_in_maps(pic, w1, b1, w2a, b2a, w2b, b2b)
    results, _ = run(in_maps)

    mid = np.empty((B, 400, H, W), dtype=np.float32)
    for core in range(8):
        b, sl = divmod(core, 4)
        h0 = sl * SLAB
        mid[b, :, h0:h0 + SLAB, :] = results[core]["out"].reshape(
            400, SLAB, W)
    # pixel shuffle r=4, then split 25 -> 5x5
    y = mid.reshape(B, 25, 4, 4, H, W).transpose(0, 1, 4, 2, 5, 3)
    return np.ascontiguousarray(y).reshape(B, 5, 5, H * 4, W * 4)


# revision 22
# speedup vs baseline: 1.1390x; 1.1390x over previous
"""Trainium2 Bass kernel for nn_DimNet (4D-conv net + pixel shuffle).

Math: the three 4D convs collapse to 2D convs over flattened angular dims:
  conv1:  in [25, 104, 104] -> out [400, 96, 96], 9x9 kernel
  conv2a: in [25, 104, 104] -> buf [180, 100, 100] (20ch x 3x3 angular window, 5x5)
  conv2b: buf [180,100,100] -> out [400, 96, 96], 5x5 kernel
  mid = (p1 + relu-path)/2; pixel-shuffle (host side, pure data movement)

Mapping to TensorE: contraction K packed as (channel, kh-shift) on partitions
(125/128-wide), kw handled by free-dim offsets into kh-shifted input copies,
accumulated in PSUM across kw / K-chunks.

Sharding: batch (2) x spatial row-slab (4 x 24 out rows) = 8 cores. Each core
computes ALL 400 output channels (4 M-chunks of 100) for its 24 rows; conv2a
is computed only for the 28 rows the slab needs (vs 100 replicated rows in
the channel-sharded layout). Gather + pixel shuffle on host.

conv2b is split as sigma 0..127 (kh,kw via free-dim offsets into buf1; 25
matmuls at K=128) + sigma 128..179 repacked (kh,kw)-dense into 13 tiles of
two 52-lane groups each (13 matmuls), built by 25 DVE scatter copies that
overlap the conv1 phase. 38 matmuls/block vs 40 in the kh-packed layout.
"""

import os
import time

import ml_dtypes
import numpy as np

import concourse.tile as tile
from concourse import bacc, mybir

F32 = mybir.dt.float32
BF16 = mybir.dt.bfloat16

MM_DT = BF16
MM_NP = ml_dtypes.bfloat16

B = 2
H = 96
W = 96
HP = H + 8   # 104
WP = W + 8   # 104
SLAB = 24    # output rows per core
SROWS = SLAB + 4  # conv2a rows a slab needs

_RUNNERS = {}


def _build_nc(reps=1):
    nc = bacc.Bacc("TRN2", target_bir_lowering=False, debug=False,
                   enable_asserts=True, num_devices=8)

    xk1 = nc.dram_tensor("xk1", [125, SROWS, WP], MM_DT, kind="ExternalInput").ap()
    # conv1 input with (cin,kh,kw) fully packed into 2025 K-lanes (zero-padded
    # to 2048 = 16 chunks of 128): xs[k, c, l, w] = xpad[cin, h0+l+kh, w+kw]
    # for lane g = 128c + k = cin*81 + kh*9 + kw
    xs = nc.dram_tensor("xs", [128, 16, SLAB, W], MM_DT, kind="ExternalInput").ap()
    w1x = nc.dram_tensor("w1x", [128, 16, 400], MM_DT, kind="ExternalInput").ap()
    w2a = nc.dram_tensor("w2a", [125, 5, 180], MM_DT, kind="ExternalInput").ap()
    w2b1 = nc.dram_tensor("w2b1", [128, 25, 400], MM_DT, kind="ExternalInput").ap()
    # conv2b sigma-chunk2 (52 wide) repacked with (kh,kw) dense: tile t holds
    # khw groups 2t (lanes 0-51) and 2t+1 (lanes 64-115, 64-aligned so DVE
    # partition bases stay 32-aligned); pad lanes have zero weights.
    # 25 khw groups -> 13 tiles -> 13 matmuls (vs 15 kh-packed).
    w2bk = nc.dram_tensor("w2bk", [13, 128, 400], MM_DT, kind="ExternalInput").ap()
    ba1 = nc.dram_tensor("ba1", [128, 1], F32, kind="ExternalInput").ap()
    ba2 = nc.dram_tensor("ba2", [52, 1], F32, kind="ExternalInput").ap()
    b1h = nc.dram_tensor("b1h", [100, 4], F32, kind="ExternalInput").ap()
    b2bh = nc.dram_tensor("b2bh", [100, 4], F32, kind="ExternalInput").ap()
    # one output per rep so no rep's work is dead (reps>1 is timing-only)
    outs_d = [nc.dram_tensor("out" if r == 0 else f"out{r}",
                             [4, 100, SLAB, W], F32,
                             kind="ExternalOutput").ap() for r in range(reps)]

    Relu = mybir.ActivationFunctionType.Relu
    Add = mybir.AluOpType.add

    def mm(out, lhsT, rhs, start, stop):
        nc.tensor.matmul(out, lhsT, rhs, start=start, stop=stop)

    from contextlib import ExitStack

    with tile.TileContext(nc) as tc:
        with (
            tc.tile_pool(name="const", bufs=1) as const,
            tc.tile_pool(name="tmp", bufs=3) as tmp,
            tc.tile_pool(name="outp", bufs=3) as outp,
        ):
          # weights/biases loaded once (shared across timing reps); ordered so
          # the tensors needed first arrive first (conv2a -> conv1 -> conv2b)
          w2a_t = const.tile([125, 5, 180], MM_DT)
          w1x_t = const.tile([128, 16, 400], MM_DT)
          w2b1_t = const.tile([128, 25, 400], MM_DT)
          w2bk_t = [const.tile([128, 400], MM_DT, name=f"w2bk{t}",
                               tag=f"w2bk{t}") for t in range(13)]
          ba1_t = const.tile([128, 1], F32)
          ba2_t = const.tile([52, 1], F32)
          b1h_t = const.tile([100, 4], F32)
          b2bh_t = const.tile([100, 4], F32)
          for t, src in ((w2a_t, w2a), (ba1_t, ba1), (ba2_t, ba2),
                         (w1x_t, w1x), (b1h_t, b1h),
                         (w2b1_t, w2b1), (b2bh_t, b2bh)):
              nc.gpsimd.dma_start(out=t[:], in_=src)
          for t in range(13):
              nc.gpsimd.dma_start(out=w2bk_t[t][:], in_=w2bk[t])

          # (kh,kw)-shifted copies of buf2: tile t lane j*52+s' holds
          # buf2[s', l+kh, w+kw] for khw group 2t+j. Pad lanes zeroed once
          # -- 0 * garbage could be NaN.
          bk3_t = [const.tile([128, SLAB, W], MM_DT, name=f"bk3_{t}",
                              tag=f"bk3_{t}") for t in range(13)]
          for bt in bk3_t:
              nc.gpsimd.memset(bt[:], 0.0)

          for _rep in range(reps):
            out = outs_d[_rep]
            _ph_a = ExitStack()
            xk1p = _ph_a.enter_context(
                tc.tile_pool(name=f"xk1p{_rep}", bufs=2))
            xsp = _ph_a.enter_context(
                tc.tile_pool(name=f"xsp{_rep}", bufs=2))
            psa = _ph_a.enter_context(
                tc.tile_pool(name=f"psa{_rep}", bufs=3, space="PSUM"))
            ps1p = _ph_a.enter_context(
                tc.tile_pool(name=f"ps1{_rep}", bufs=3, space="PSUM"))
            buf1_t = const.tile([128, SROWS, 100], MM_DT)
            buf2_t = const.tile([52, SROWS, 100], MM_DT)
            # conv1 result staged in bf16 to fit the xs stream in SBUF
            p1h_t = const.tile([100, 4, SLAB, W], MM_DT)

            # conv2a input slab in one DMA (0.73 MB)
            xc1 = xk1p.tile([125, SROWS, WP], MM_DT)
            nc.sync.dma_start(out=xc1[:], in_=xk1)
            # first conv1 xs block prefetched during conv2a
            xs_blocks = []
            for l0 in range(0, SLAB, 5):
                rb = min(5, SLAB - l0)
                xst = xsp.tile([128, 16, 5, W], MM_DT, tag="xs",
                               name=f"xst{l0}")
                xs_blocks.append((l0, rb, xst))
            l0, rb, xst = xs_blocks[0]
            nc.sync.dma_start(out=xst[:, :, 0:rb, :],
                              in_=xs[:, :, l0:l0 + rb, :])

            # ---- Phase A1: conv2a over the 28 slab rows (blocks of 5+3)
            for l0 in range(0, SROWS, 5):
                rb = min(5, SROWS - l0)
                pa1 = psa.tile([128, 5, 100], F32, tag="pa")
                for kw in range(5):
                    mm(pa1[:, 0:rb, :], w2a_t[:, kw, 0:128],
                       xc1[:, l0:l0 + rb, kw:kw + 100],
                       start=(kw == 0), stop=(kw == 4))
                nc.scalar.activation(buf1_t[:, l0:l0 + rb, :], pa1[:, 0:rb, :],
                                     Relu, bias=ba1_t[:])
                pa2 = psa.tile([52, 5, 100], F32, tag="pa")
                for kw in range(5):
                    mm(pa2[:, 0:rb, :], w2a_t[:, kw, 128:180],
                       xc1[:, l0:l0 + rb, kw:kw + 100],
                       start=(kw == 0), stop=(kw == 4))
                # chunk2 relu on DVE so ACT isn't the drain bottleneck
                nc.vector.tensor_scalar(buf2_t[:, l0:l0 + rb, :],
                                        pa2[:, 0:rb, :],
                                        ba2_t[:], 0.0, Add,
                                        mybir.AluOpType.max)

            # ---- Phase A2: conv1, blocks (5,5,5,5,4) x 4 M-chunks x 16
            # K-chunks; xs blocks double-buffered; the 25 bk3 scatter copies
            # (DVE) are spread across blocks so they overlap conv1 matmuls
            # without starving the ps1 drains
            for bi, (l0, rb, xst) in enumerate(xs_blocks):
                if bi + 1 < len(xs_blocks):
                    n_l0, n_rb, n_xst = xs_blocks[bi + 1]
                    nc.sync.dma_start(out=n_xst[:, :, 0:n_rb, :],
                                      in_=xs[:, :, n_l0:n_l0 + n_rb, :])
                for g in range(bi * 5, bi * 5 + 5):
                    kh, kw = divmod(g, 5)
                    t, j = divmod(g, 2)
                    nc.vector.tensor_copy(
                        bk3_t[t][j * 64:j * 64 + 52, :, :],
                        buf2_t[:, kh:kh + SLAB, kw:kw + W])
                for mc in range(4):
                    co0 = mc * 100
                    p1 = ps1p.tile([100, 5, W], F32, tag="p1")
                    for c in range(16):
                        mm(p1[:, 0:rb, :], w1x_t[:, c, co0:co0 + 100],
                           xst[:, c, 0:rb, :],
                           start=(c == 0), stop=(c == 15))
                    # w1/b1 pre-halved on host: p1h = psum + b1h
                    nc.vector.tensor_scalar_add(
                        p1h_t[:, mc, l0:l0 + rb, :], p1[:, 0:rb, :],
                        b1h_t[:, mc:mc + 1])

            # phase-A psum/xk pools released -> conv2b gets 6 PSUM banks
            _ph_a.close()
            ps2p = ExitStack()
            ps2 = ps2p.enter_context(
                tc.tile_pool(name=f"ps2{_rep}", bufs=6, space="PSUM"))

            # ---- Phase B: conv2b, 4 M-chunks x blocks (5,5,5,5,4) + merge
            for mc in range(4):
                co0 = mc * 100
                for l0 in range(0, SLAB, 5):
                    rb = min(5, SLAB - l0)
                    p2 = ps2.tile([100, 5, W], F32, tag="p2")
                    p2v = p2[:, 0:rb, :]
                    first = True
                    for kh in range(5):
                        for kw in range(5):
                            mm(p2v, w2b1_t[:, kh * 5 + kw, co0:co0 + 100],
                               buf1_t[:, l0 + kh:l0 + kh + rb, kw:kw + W],
                               start=first, stop=False)
                            first = False
                    for t in range(13):
                        mm(p2v, w2bk_t[t][:, co0:co0 + 100],
                           bk3_t[t][:, l0:l0 + rb, :],
                           start=False, stop=(t == 12))
                    # w2b/b2b pre-halved on host:
                    # relu(conv2b+b2b)/2 = relu(psum+b2bh)
                    tt = tmp.tile([100, 5, W], F32)
                    nc.scalar.activation(tt[:, 0:rb, :], p2v, Relu,
                                         bias=b2bh_t[:, mc:mc + 1])
                    ot = outp.tile([100, 5, W], F32)
                    nc.vector.tensor_add(ot[:, 0:rb, :], tt[:, 0:rb, :],
                                         p1h_t[:, mc, l0:l0 + rb, :])
                    nc.scalar.dma_start(out=out[mc, :, l0:l0 + rb, :],
                                        in_=ot[:, 0:rb, :])
            ps2p.close()

    nc.compile()
    return nc


def _w2bk13(W2B):
    """[13, 128, 400] (kh,kw)-dense repack of W2B[128:]: tile t lane j*52+s'
    = W2B[128+s', khw=2t+j, :]; pad lanes zero."""
    src = W2B[128:]  # [52, 25, 400]
    out = np.zeros((13, 128, 400), dtype=np.float32)
    for g in range(25):
        t, j = divmod(g, 2)
        out[t, j * 64:j * 64 + 52] = src[:, g, :]
    return np.ascontiguousarray(out.astype(MM_NP))


def _prep_in_maps(pic, w1, b1, w2a, b2a, w2b, b2b):
    pic = np.asarray(pic, dtype=np.float32).reshape(B, 25, H, W)
    w1r = np.asarray(w1, dtype=np.float32).reshape(400, 25, 9, 9)
    b1 = np.asarray(b1, dtype=np.float32)
    w2a = np.asarray(w2a, dtype=np.float32)
    b2a = np.asarray(b2a, dtype=np.float32)
    w2b = np.asarray(w2b, dtype=np.float32)
    b2b = np.asarray(b2b, dtype=np.float32)

    xpad = np.full((B, 25, HP, WP), 0.5, dtype=np.float32)
    xpad[:, :, 4:4 + H, 4:4 + W] = pic
    # xk1[b, cin*5+kh, r, w] = xpad[b, cin, r+kh, w]   (kh 0..4, r 0..99)
    xk1 = np.stack([xpad[:, :, kh:kh + 100, :] for kh in range(5)],
                   axis=2).reshape(B, 125, 100, WP)
    xk1 = xk1.astype(MM_NP)
    # conv1 (cin,kh,kw)-dense input, lane g = cin*81 + kh*9 + kw, zero-padded
    # to 2048 lanes and laid out [k=128, c=16, r, w] to match the SBUF tiles
    sview = np.lib.stride_tricks.as_strided
    xs_full = []
    for b in range(B):
        xp = np.ascontiguousarray(xpad[b])
        s1, s2, s3 = xp.strides
        XS = sview(xp, (25, 9, 9, H, W), (s1, s2, s3, s2, s3))
        XSP = np.zeros((2048, H, W), dtype=np.float32)
        XSP[:2025] = XS.reshape(2025, H, W)
        xs_full.append(np.ascontiguousarray(
            XSP.reshape(16, 128, H, W).transpose(1, 0, 2, 3)).astype(MM_NP))

    # W2A[p=(a1*5+a2)*5+kh, kw, m=a1'*60+a2'*20+c] = w2a[c,0,da1,da2,kh,kw]
    W2A = np.zeros((125, 5, 180), dtype=np.float32)
    for a1p in range(3):
        for a2p in range(3):
            m0 = a1p * 60 + a2p * 20
            for da1 in range(3):
                for da2 in range(3):
                    p0 = ((a1p + da1) * 5 + (a2p + da2)) * 5
                    W2A[p0:p0 + 5, :, m0:m0 + 20] = np.transpose(
                        w2a[:, 0, da1, da2, :, :], (1, 2, 0))
    ba_full = np.tile(b2a, 9).astype(np.float32)[:, None]  # [180,1]
    W2A = np.ascontiguousarray(W2A.astype(MM_NP))

    # weights shared by all cores; w1, w2b (and biases) pre-scaled by 0.5 so
    # the (p1+p2)/2 average is folded into the matmuls.
    w1s = 0.5 * w1r  # [400, 25, 9, 9]
    W1P = np.zeros((2048, 400), dtype=np.float32)
    W1P[:2025] = np.transpose(w1s, (1, 2, 3, 0)).reshape(2025, 400)
    W1X = np.ascontiguousarray(
        W1P.reshape(16, 128, 400).transpose(1, 0, 2)).astype(MM_NP)
    w2bs = 0.5 * w2b  # [400, 20, 3, 3, 5, 5]
    W2B = np.ascontiguousarray(
        np.transpose(w2bs, (2, 3, 1, 4, 5, 0)).reshape(180, 25, 400))
    w2b1_h = np.ascontiguousarray(W2B[:128].astype(MM_NP))
    w2bk_h = _w2bk13(W2B)
    ba1_h = np.ascontiguousarray(ba_full[:128])
    ba2_h = np.ascontiguousarray(ba_full[128:])
    b1h_h = np.ascontiguousarray((0.5 * b1).reshape(4, 100).T)
    b2bh_h = np.ascontiguousarray((0.5 * b2b).reshape(4, 100).T)

    in_maps = []
    for core in range(8):
        b, sl = divmod(core, 4)
        h0 = sl * SLAB
        in_maps.append({
            "xk1": np.ascontiguousarray(xk1[b][:, h0:h0 + SROWS, :]),
            "xs": np.ascontiguousarray(xs_full[b][:, :, h0:h0 + SLAB, :]),
            "w1x": W1X,
            "w2a": W2A,
            "w2b1": w2b1_h,
            "w2bk": w2bk_h,
            "ba1": ba1_h,
            "ba2": ba2_h,
            "b1h": b1h_h,
            "b2bh": b2bh_h,
        })
    return in_maps


def _get_runner(reps=1):
    """Build nc once per reps and return a cached jitted SPMD executor."""
    if reps in _RUNNERS:
        return _RUNNERS[reps]

    import jax
    from jax.experimental.shard_map import shard_map
    from jax.sharding import Mesh, NamedSharding, PartitionSpec

    from concourse import mybir as _mybir
    from concourse.bass2jax import (_bass_exec_p, install_neuronx_cc_hook,
                                    partition_id_tensor)

    nc = _build_nc(reps)
    install_neuronx_cc_hook()

    n_cores = 8
    partition_name = (nc.partition_id_tensor.name
                      if nc.partition_id_tensor else None)
    in_names, out_names, out_avals, zero_outs = [], [], [], []
    for alloc in nc.m.functions[0].allocations:
        if not isinstance(alloc, _mybir.MemoryLocationSet):
            continue
        name = alloc.memorylocations[0].name
        if alloc.kind == "ExternalInput":
            if name != partition_name:
                in_names.append(name)
        elif alloc.kind == "ExternalOutput":
            shape = tuple(alloc.tensor_shape)
            dtype = _mybir.dt.np(alloc.dtype)
            out_names.append(name)
            out_avals.append(jax.core.ShapedArray(shape, dtype))
            zero_outs.append(np.zeros((n_cores * shape[0],) + shape[1:], dtype))
    assert nc.dbg_addr is None
    n_params = len(in_names)
    all_names = in_names + out_names
    if partition_name is not None:
        all_names = all_names + [partition_name]

    def _body(*args):
        operands = list(args)
        if partition_name is not None:
            operands.append(partition_id_tensor())
        outs = _bass_exec_p.bind(
            *operands,
            out_avals=tuple(out_avals),
            in_names=tuple(all_names),
            out_names=tuple(out_names),
            lowering_input_output_aliases=(),
            sim_require_finite=True,
            sim_require_nnan=True,
            nc=nc,
        )
        return tuple(outs)

    devices = jax.devices()[:n_cores]
    mesh = Mesh(np.asarray(devices), ("core",))
    nspec = (PartitionSpec("core"),) * (n_params + len(out_names))
    sharded = jax.jit(
        shard_map(_body, mesh=mesh, in_specs=nspec,
                  out_specs=(PartitionSpec("core"),) * len(out_names)),
        keep_unused=True)
    sharding = NamedSharding(mesh, PartitionSpec("core"))

    class Runner:
        def put(self, in_maps):
            """Transfer inputs (+ zero output bufs) to the devices once."""
            concat_in = [
                np.concatenate([np.asarray(m[name]) for m in in_maps], axis=0)
                for name in in_names
            ]
            return [jax.device_put(x, sharding)
                    for x in concat_in + zero_outs]

        def exec_timed(self, dev_args):
            t0 = time.perf_counter()
            out_arrs = sharded(*dev_args)
            # one sync only: under axon each block_until_ready is a costly
            # RPC, and blocking any output waits for the whole execution
            out_arrs[0].block_until_ready()
            return out_arrs, time.perf_counter() - t0

        def dispatch(self, dev_args):
            """Async dispatch without blocking (for pipelined timing)."""
            return sharded(*dev_args)

        def __call__(self, in_maps):
            out_arrs, dt = self.exec_timed(self.put(in_maps))
            per_core = [
                {name: np.asarray(out_arrs[i]).reshape(
                    n_cores, *out_avals[i].shape)[c]
                 for i, name in enumerate(out_names)}
                for c in range(n_cores)
            ]
            return per_core, dt

    run = Runner()
    _RUNNERS[reps] = run
    return run


def kernel(pic, w1, b1, w2a, b2a, w2b, b2b):
    run = _get_runner()
    in_maps = _prep_in_maps(pic, w1, b1, w2a, b2a, w2b, b2b)
    results, _ = run(in_maps)

    mid = np.empty((B, 400, H, W), dtype=np.float32)
    for core in range(8):
        b, sl = divmod(core, 4)
        h0 = sl * SLAB
        mid[b, :, h0:h0 + SLAB, :] = results[core]["out"].reshape(
            400, SLAB, W)
    # pixel shuffle r=4, then split 25 -> 5x5
    y = mid.reshape(B, 25, 4, 4, H, W).transpose(0, 1, 4, 2, 5, 3)
    return np.ascontiguousarray(y).reshape(B, 5, 5, H * 4, W * 4)
